# revision 104
# baseline (speedup 1.0000x reference)
"""Trainium2 Bass kernel for the masked-bottleneck + coord-attention block.

Sharding: data-parallel over batch (B=16 -> 8 cores x 2 samples), weights
replicated. Everything below runs per-core on its 2-sample shard.

Per-sample dataflow (channels on partitions, 512 = 4 chunks of 128):
  s_logits   : PE matmul (fp32 - the mask threshold margins are ~1e-5 sigma,
               fp32r's ~1e-4 noise flips pixels) with sm_w replicated across
               all 128 output columns, so the [1,HW] logit row lands
               replicated on all partitions.
  x -> xr    : ACT Copy converts the fp32 x chunks to float32r for the value
               path; the per-chunk GAP sums ride the fp32-exact accum_out
               side-channel for free (feeds cm1).
  signmask   : DVE is_gt -> {0,1} replicated mask. Dilated mask built on a
               [58,58] 2D view via tiny TT-max ops + partition-shift DMAs,
               then GPSIMD partition_broadcast back to [128,HW].
  conv1/2/3  : float32r matmuls (1 PE cycle/row vs 4 for fp32; ~13-bit reads
               are fine for the value path). Channel-mask logits stay exact:
               the pooled sums come from fp32 accum_out side-channels, and the
               tiny cm matmuls run in fp32. cm1/cm2 are per-channel {0,1}
               masks, so they commute through the next conv's contraction:
               they are folded into scaled stationaries (w2s = w2T*cm1,
               w3s = w3T*cm2) + pooled-vector scalings instead of gating
               h1m/h2m elementwise - this removes the pooled-mask serial
               dependency from the conv pipelines.
  conv3+CA   : pools of h3 derived by linearity (W3 @ pool(h2m) + b3*pool(s)),
               so h3 is never materialized: conv3 psum -> DVE *ah*cm3 -> DVE
               *aw -> PE accumulates idn@xr (residual) -> ACT Relu -> out.
"""

import os
import sys

for _p in ("/opt/trn_rl_repo", os.path.expanduser("~/.axon_site/_ro/trn_rl_repo")):
    if os.path.isdir(_p) and _p not in sys.path:
        sys.path.insert(0, _p)

import numpy as np
from contextlib import ExitStack

import concourse.bass as bass
from concourse import bacc
import concourse.mybir as mybir
import concourse.tile as tile
from concourse import library_config
from concourse.bass_utils import run_bass_kernel_spmd

f32 = mybir.dt.float32
f32r = mybir.dt.float32r
AF = mybir.ActivationFunctionType
OP = mybir.AluOpType
AX = mybir.AxisListType

NCORES = 8
BS = 2                  # samples per core
KC = 4                  # 512 input channels -> 4 chunks of 128
MC = 4                  # 512 output channels -> 4 chunks of 128
WID = 128
MIP = 16
H = W = 56
NPIX = H * W            # 3136
PW = 58                 # zero-padded 2D side
PADN = PW * PW          # 3364
TW = 448                # slog/conv1/conv2 N-tile (8 rows)
NT = NPIX // TW         # 7
RT = TW // W            # 8 rows per tile
# stage I (conv3/CA/residual) tiling: 784 cols = 2 halves of 392 (7 rows each)
IW = 784
NIT = NPIX // IW        # 4
IH = 392                # half width (7 rows)
IR = 7                  # rows per half


KREPEAT = int(os.environ.get("KREPEAT", "1"))
KSTAGE = int(os.environ.get("KSTAGE", "9"))

# DRAM dtypes: f32r for weights only consumed by float32r matmuls (np maps
# float32r -> np.float32, so host arrays are unchanged).
# All weights ship in two packed SBUF blobs (one f32, one float32r) so the
# whole preload is 2 DMAs instead of ~25 (dispatch cost + queue contention).
# float32r is used only for the big-N matmuls (conv1/2/3, conv3-bias,
# residual): small-N f32r matmuls fail the walrus ISA check and have no
# speed benefit anyway. w3T/b3r have a second f32 copy for the stage-H
# pools. zpad is a zero region (f32r tiles cannot be memset - walrus ISA
# check - so zero fills are DMA'd from it).
WBR_LAYOUT = [
    ("w1T", (128, KC, 128)), ("w2T", (128, 9, 128)), ("w3T", (128, MC, 128)),
    ("idn", (128, 128)), ("b3r", (1, 512)), ("zpad", (128, PW)),
]
WBF_LAYOUT = [
    ("w3Tf", (128, MC, 128)), ("smw", (128, KC)), ("cm1w", (128, KC, 128)),
    ("cm2w", (128, 128)), ("cm3w", (128, MC, 128)), ("b3rf", (1, 512)),
    ("caw1", (128, KC, MIP)), ("cawh", (MIP, MC, 128)), ("caww", (MIP, MC, 128)),
    ("b1v", (128, 1)), ("b2v", (128, 1)), ("smbneg", (128, 1)),
    ("cm1nb", (128, 1)), ("cm2nb", (128, 1)), ("cm3nb", (128, MC)),
    ("cab1p3", (MIP, 1)), ("cabh", (128, MC)), ("cabw", (128, MC)),
]


def _blob_cols(layout):
    return sum(int(np.prod(s[1:])) for _, s in layout)


CR = _blob_cols(WBR_LAYOUT)
CF = _blob_cols(WBF_LAYOUT)


def _emit(nc, tc, ctx, d):
    sing = ctx.enter_context(tc.tile_pool(name="sing", bufs=1))
    xcp = ctx.enter_context(tc.tile_pool(name="xcp", bufs=4))
    xslp = ctx.enter_context(tc.tile_pool(name="xslp", bufs=8))
    big1 = ctx.enter_context(tc.tile_pool(name="big1", bufs=1))
    mid = ctx.enter_context(tc.tile_pool(name="mid", bufs=2))
    hrp = ctx.enter_context(tc.tile_pool(name="hrp", bufs=4))
    wsp = ctx.enter_context(tc.tile_pool(name="wsp", bufs=1))
    outp = ctx.enter_context(tc.tile_pool(name="outp", bufs=2))
    sm2 = ctx.enter_context(tc.tile_pool(name="sm2", bufs=2))
    dil1 = ctx.enter_context(tc.tile_pool(name="dil1", bufs=1))
    pbig = ctx.enter_context(tc.tile_pool(name="pbig", bufs=2, space="PSUM"))
    pc3 = ctx.enter_context(tc.tile_pool(name="pc3", bufs=2, space="PSUM"))
    pvec = ctx.enter_context(tc.tile_pool(name="pvec", bufs=2, space="PSUM"))

    nc.gpsimd.load_library(library_config.mlp)

    # ---- weights / constants: 2 packed blob DMAs (gpsimd path so they do
    # not queue ahead of the first sample's x slices on the HWDGE queues) ----
    wbr = sing.tile([128, CR], f32r, name="wbr")
    nc.gpsimd.dma_start(out=wbr, in_=d["wbr"].ap())
    wbf = sing.tile([128, CF], f32, name="wbf")
    nc.gpsimd.dma_start(out=wbf, in_=d["wbf"].ap())

    def _views(blob, layout):
        vs, off = {}, 0
        for name, shp in layout:
            n = int(np.prod(shp[1:]))
            v = blob[0:shp[0], off:off + n]
            if len(shp) == 3:
                v = v.rearrange("p (a b) -> p a b", a=shp[1])
            vs[name] = v
            off += n
        return vs

    V = _views(wbr, WBR_LAYOUT)
    V.update(_views(wbf, WBF_LAYOUT))
    w1T, w2T, w3T, idn, caw1 = V["w1T"], V["w2T"], V["w3T"], V["idn"], V["caw1"]
    cawh, caww, b3r = V["cawh"], V["caww"], V["b3r"]
    w3Tf, smw, cm1w, cm2w, cm3w = V["w3Tf"], V["smw"], V["cm1w"], V["cm2w"], V["cm3w"]
    b3rf, b1v, b2v, smbneg = V["b3rf"], V["b1v"], V["b2v"], V["smbneg"]
    cm1nb, cm2nb, cm3nb, cab1p3 = V["cm1nb"], V["cm2nb"], V["cm3nb"], V["cab1p3"]
    cabh, cabw = V["cabh"], V["cabw"]
    zpad_r = V["zpad"]               # [128, PW] zeros, f32r
    ones58 = sing.tile([PW, 1], f32, name="ones58")
    nc.vector.memset(ones58, 1.0)

    x_d = d["x"]
    out_d = d["out"]

    # f32r scratch tiles, ring-zeroed ONCE: each sample fully rewrites the
    # interiors, so the zero borders survive across samples/repeats.
    h1m = big1.tile([128, PADN], f32r, name="h1m", tag="h1m")
    h1m3 = h1m.rearrange("p (r c) -> p r c", r=PW)
    # only the zero-padding ring needs filling - the interior is fully
    # rewritten by every sample's conv1 mask-apply before conv2 reads it
    nc.gpsimd.dma_start(out=h1m3[:, 0, :], in_=zpad_r)
    nc.gpsimd.dma_start(out=h1m3[:, PW - 1, :], in_=zpad_r)
    nc.gpsimd.dma_start(out=h1m3[:, :, 0], in_=zpad_r)
    nc.gpsimd.dma_start(out=h1m3[:, :, PW - 1], in_=zpad_r)
    t2d = dil1.tile([PW, PW], f32r, name="t2d", tag="t2d")
    hm1 = dil1.tile([PW, PW], f32r, name="hm1", tag="hm1")
    hm2 = dil1.tile([PW, PW], f32r, name="hm2", tag="hm2")
    vup = dil1.tile([PW, PW], f32r, name="vup", tag="vup")
    vdn = dil1.tile([PW, PW], f32r, name="vdn", tag="vdn")
    dl1 = dil1.tile([PW, PW], f32r, name="dl1", tag="dl1")
    dl2 = dil1.tile([PW, PW], f32r, name="dl2", tag="dl2")
    for t in (t2d, hm1, hm2, vup, vdn):
        nc.gpsimd.dma_start(out=t, in_=zpad_r[0:PW, :])

    for rep in range(KREPEAT):
      for s in range(BS):
        # ---- stage A+B interleaved: stream x in [128,TW] slices; each slice
        # feeds (a) the fp32 slog matmul (M=1 logit row in h2m row 0 - the
        # mask threshold margins are ~1e-5 sigma, f32r noise would flip
        # pixels) and (b) the ACT f32->f32r conversion into the persistent
        # xkr chunks, whose fp32-exact accum_out side-channel yields the GAP
        # partials for cm1. Slices start compute ~2.5us after the first DMA
        # instead of waiting for the full 12.8MB x load. ----
        xkr = []
        for k in range(KC):
            xrt = xcp.tile([128, NPIX], f32r, name=f"xr_s{s}k{k}", tag="xcr")
            xkr.append(xrt)
        NP2 = (NT + 1) // 2          # tile pairs (last pair is a single tile)
        pxp = sm2.tile([128, KC * NP2], f32, name=f"pxp{s}", tag="pxp")
        h2m = big1.tile([128, NPIX], f32r, name=f"h2m{s}", tag="h2m")
        signmask = big1.tile([128, NPIX], f32r, name=f"signmask{s}", tag="signmask")
        for p in range(NP2):
            tlo = 2 * p
            ntl = min(2, NT - tlo)
            w = ntl * TW
            xsl = []
            for k in range(KC):
                sl = xslp.tile([128, 2 * TW], f32, name=f"xsl{s}_{p}_{k}", tag="xsl")
                nc.sync.dma_start(
                    out=sl[:, 0:w],
                    in_=x_d[s, 128 * k:128 * (k + 1), RT * tlo:RT * (tlo + ntl), :]
                    .rearrange("c h w -> c (h w)"))
                xsl.append(sl)
            for tt in range(ntl):
                t = tlo + tt
                ps = pbig.tile([128, TW], f32, name=f"ps_slog{s}_{t}", tag="pbig")
                for k in range(KC):
                    nc.tensor.matmul(ps[0:1, :], smw[:, k:k + 1],
                                     xsl[k][:, TW * tt:TW * (tt + 1)],
                                     start=(k == 0), stop=(k == KC - 1))
                nc.vector.tensor_scalar(out=h2m[0:1, TW * t:TW * (t + 1)],
                                        in0=ps[0:1, :], scalar1=smbneg[0:1, :],
                                        scalar2=None, op0=OP.is_gt)
            for k in range(KC):
                nc.scalar.activation(out=xkr[k][:, TW * tlo:TW * (tlo + ntl)],
                                     in_=xsl[k][:, 0:w], func=AF.Copy,
                                     accum_out=pxp[:, k * NP2 + p:k * NP2 + p + 1])
        nc.gpsimd.partition_broadcast(signmask[:, :], h2m[0:1, :])

        if KSTAGE < 2:
            continue
        # ---------------- stage C: cm1 from pooled x ----------------
        px = sm2.tile([128, KC], f32, name=f"px{s}", tag="px")
        nc.vector.tensor_reduce(out=px, in_=pxp.rearrange("p (k q) -> p k q", k=KC),
                                axis=AX.X, op=OP.add)
        pl1 = pvec.tile([128, 1], f32, name=f"pl1{s}", tag="pvec")
        for k in range(KC):
            nc.tensor.matmul(pl1[:, :], cm1w[:, k, :], px[:, k:k + 1],
                             start=(k == 0), stop=(k == KC - 1))
        cm1 = sm2.tile([128, 1], f32, name=f"cm1{s}", tag="cm1")
        nc.vector.tensor_scalar(out=cm1, in0=pl1[:, :], scalar1=cm1nb[:, :],
                                scalar2=None, op0=OP.is_gt)
        # cm1 is per-channel, so it commutes through conv2's contraction:
        # fold it into a scaled conv2 stationary (w2s = w2T * cm1) and into
        # the pooled sum (p1s * cm1) instead of gating h1m elementwise. This
        # removes cm1 from conv1's epilogue critical path entirely.
        w2s = wsp.tile([128, 9, 128], f32r, name=f"w2s{s}", tag="w2s")
        nc.vector.tensor_scalar(out=w2s, in0=w2T, scalar1=cm1[:, :],
                                scalar2=None, op0=OP.mult)

        if KSTAGE < 3:
            continue
        # ---------------- stage D: dilated mask ----------------
        nc.sync.dma_start(out=t2d[1:57, 1:57], in_=h2m[0:1, :])
        nc.vector.tensor_tensor(out=hm1[:, 1:57], in0=t2d[:, 0:56], in1=t2d[:, 2:58],
                                op=OP.max)
        nc.vector.tensor_tensor(out=hm2[:, 1:57], in0=hm1[:, 1:57], in1=t2d[:, 1:57],
                                op=OP.max)
        nc.sync.dma_start(out=vup[0:57, 1:57], in_=hm2[1:58, 1:57])
        nc.scalar.dma_start(out=vdn[1:58, 1:57], in_=hm2[0:57, 1:57])
        nc.vector.tensor_tensor(out=dl1, in0=hm2, in1=vup, op=OP.max)
        nc.vector.tensor_tensor(out=dl2, in0=dl1, in1=vdn, op=OP.max)
        # borrow h2m row 0 again for the dilated-mask staging row
        nc.sync.dma_start(out=h2m[0:1, :], in_=dl2[1:57, 1:57])
        sdil = big1.tile([128, NPIX], f32r, name=f"sdil{s}", tag="sdil")
        # per-tile broadcast chunks so conv1's first mask-apply starts
        # ~0.6us after the staging row lands instead of ~4.5us
        for t in range(NT):
            nc.gpsimd.partition_broadcast(sdil[:, TW * t:TW * (t + 1)],
                                          h2m[0:1, TW * t:TW * (t + 1)])

        if KSTAGE < 4:
            continue
        # ---------------- stage E: conv1 ----------------
        ph1 = sm2.tile([128, NT + 1], f32, name=f"ph1_{s}", tag="ph1_")
        for t in range(NT):
            ps = pbig.tile([128, TW], f32, name=f"ps_c1_{s}_{t}", tag="pbig")
            for k in range(KC):
                nc.tensor.matmul(ps[:, :], w1T[:, k, :], xkr[k][:, TW * t:TW * (t + 1)],
                                 start=(k == 0), stop=(k == KC - 1))
            h1r = hrp.tile([128, TW], f32, name=f"h1r{s}_{t}", tag="hr")
            nc.scalar.activation(out=h1r, in_=ps[:, :], func=AF.Relu,
                                 bias=b1v[:, :])
            # h1m is cm1-UNGATED (cm1 folded into w2s / p1s) - only sdil here
            nc.vector.scalar_tensor_tensor(
                out=h1m3[:, 1 + RT * t:1 + RT * (t + 1), 1:57],
                in0=sdil[:, TW * t:TW * (t + 1)].rearrange("p (a b) -> p a b", a=RT),
                scalar=1.0,
                in1=h1r.rearrange("p (a b) -> p a b", a=RT),
                op0=OP.mult, op1=OP.mult,
                accum_out=ph1[:, t:t + 1])
        # ph1 pooling (DVE - emitted before conv2's STTs so it runs as soon
        # as conv1's accums land; the dependent pl2 matmul is emitted AFTER
        # conv2's matmuls so it never stalls the in-order PE queue)
        p1r = sm2.tile([128, 1], f32, name=f"p1r{s}", tag="p1r")
        nc.vector.tensor_reduce(out=p1r, in_=ph1[:, 0:NT], axis=AX.X, op=OP.add)
        p1s = sm2.tile([128, 1], f32, name=f"p1s{s}", tag="p1s")
        nc.vector.tensor_scalar(out=p1s, in0=p1r, scalar1=cm1[:, :],
                                scalar2=None, op0=OP.mult)
        # DVE-side producers for the post-conv2 psx matmul, emitted here so
        # they don't queue behind conv2's mask-applies on the DVE
        syc = sm2.tile([PW, 1], f32, name=f"syc{s}", tag="syc")
        nc.vector.tensor_reduce(out=syc, in_=t2d, axis=AX.X, op=OP.add)
        t2df = dil1.tile([PW, PW], f32, name=f"t2df{s}", tag="t2df")
        nc.vector.tensor_copy(out=t2df, in_=t2d)

        if KSTAGE < 5:
            continue
        # ---------------- stage G: conv2 ----------------
        ph2 = sm2.tile([128, NT + 1], f32, name=f"ph2_{s}", tag="ph2_")
        for t in range(NT):
            ps = pbig.tile([128, TW], f32, name=f"ps_c2_{s}_{t}", tag="pbig")
            first = True
            for dy in range(3):
                for dx in range(3):
                    nc.tensor.matmul(
                        ps[:, :], w2s[:, 3 * dy + dx, :],
                        h1m3[:, RT * t + dy:RT * t + dy + RT, dx:dx + 56],
                        start=first, stop=(dy == 2 and dx == 2))
                    first = False
            h2r = hrp.tile([128, TW], f32, name=f"h2r{s}_{t}", tag="hr")
            nc.scalar.activation(out=h2r, in_=ps[:, :], func=AF.Relu,
                                 bias=b2v[:, :])
            # h2m is cm2-UNGATED (cm2 folded into w3s / p2s / CA pools)
            nc.vector.scalar_tensor_tensor(
                out=h2m[:, TW * t:TW * (t + 1)],
                in0=signmask[:, TW * t:TW * (t + 1)],
                scalar=1.0,
                in1=h2r[:, :],
                op0=OP.mult, op1=OP.mult,
                accum_out=ph2[:, t:t + 1])
        # cm2 + scaled conv3 stationary + mask row/col counts: emitted after
        # conv2's matmuls so the tiny PE ops don't stall the PE queue
        pl2 = pvec.tile([128, 1], f32, name=f"pl2{s}", tag="pvec")
        nc.tensor.matmul(pl2[:, :], cm2w[:, :], p1s[:, :], start=True, stop=True)
        cm2 = sm2.tile([128, 1], f32, name=f"cm2{s}", tag="cm2")
        nc.vector.tensor_scalar(out=cm2, in0=pl2[:, :], scalar1=cm2nb[:, :],
                                scalar2=None, op0=OP.is_gt)
        w3s = wsp.tile([128, MC, 128], f32r, name=f"w3s{s}", tag="w3s")
        nc.vector.tensor_scalar(out=w3s, in0=w3T, scalar1=cm2[:, :],
                                scalar2=None, op0=OP.mult)
        psx = pvec.tile([PW, 1], f32, name=f"psx{s}", tag="pvec")
        nc.tensor.matmul(psx[:, :], t2df[:, :], ones58[:, :], start=True, stop=True)
        sxc = sm2.tile([PW, 1], f32, name=f"sxc{s}", tag="sxc")
        nc.vector.tensor_copy(out=sxc, in_=psx[:, :])
        sy_row = sm2.tile([1, W], f32, name=f"sy_row{s}", tag="sy_row")
        sx_row = sm2.tile([1, W], f32, name=f"sx_row{s}", tag="sx_row")
        nc.sync.dma_start(out=sy_row, in_=syc[1:57, 0:1])
        nc.sync.dma_start(out=sx_row, in_=sxc[1:57, 0:1])
        p2r = sm2.tile([128, 1], f32, name=f"p2r{s}", tag="p2r")
        nc.vector.tensor_reduce(out=p2r, in_=ph2[:, 0:NT], axis=AX.X, op=OP.add)
        p2s = sm2.tile([128, 1], f32, name=f"p2s{s}", tag="p2s")
        nc.vector.tensor_scalar(out=p2s, in0=p2r, scalar1=cm2[:, :],
                                scalar2=None, op0=OP.mult)

        if KSTAGE < 6:
            continue
        # ---------------- stage H: cm3 + coord-attention vectors ----------
        cm3 = sm2.tile([128, MC], f32, name=f"cm3_{s}", tag="cm3_")
        for mc in range(MC):
            pl3 = pvec.tile([128, 1], f32, name=f"pl3{s}_{mc}", tag="pvec")
            nc.tensor.matmul(pl3[:, :], cm3w[:, mc, :], p2s[:, :], start=True, stop=True)
            nc.vector.tensor_scalar(out=cm3[:, mc:mc + 1], in0=pl3[:, :],
                                    scalar1=cm3nb[:, mc:mc + 1], scalar2=None,
                                    op0=OP.is_gt)
        xh_raw = sm2.tile([128, W], f32, name=f"xh_raw{s}", tag="xh_raw")
        xw_raw = sm2.tile([128, W], f32, name=f"xw_raw{s}", tag="xw_raw")
        nc.vector.tensor_reduce(out=xh_raw, in_=h2m.rearrange("p (y x) -> p y x", y=H),
                                axis=AX.X, op=OP.add)
        nc.vector.tensor_reduce(out=xw_raw, in_=h2m.rearrange("p (y x) -> p x y", y=H),
                                axis=AX.X, op=OP.add)
        xh_pre = sm2.tile([128, W], f32, name=f"xh_pre{s}", tag="xh_pre")
        xw_pre = sm2.tile([128, W], f32, name=f"xw_pre{s}", tag="xw_pre")
        nc.vector.tensor_scalar(out=xh_pre, in0=xh_raw, scalar1=cm2[:, :],
                                scalar2=None, op0=OP.mult)
        nc.vector.tensor_scalar(out=xw_pre, in0=xw_raw, scalar1=cm2[:, :],
                                scalar2=None, op0=OP.mult)
        xcat = sm2.tile([128, KC, 2 * W], f32, name=f"xcat{s}", tag="xcat")
        for mc in range(MC):
            pxh = pvec.tile([128, W], f32, name=f"pxh{s}_{mc}", tag="pvec")
            nc.tensor.matmul(pxh[:, :], w3Tf[:, mc, :], xh_pre[:, :], start=True, stop=False)
            nc.tensor.matmul(pxh[:, :], b3rf[0:1, 128 * mc:128 * (mc + 1)], sy_row[:, :],
                             start=False, stop=True)
            nc.scalar.activation(out=xcat[:, mc, 0:W], in_=pxh[:, :], func=AF.Copy,
                                 scale=cm3[:, mc:mc + 1])
            pxw = pvec.tile([128, W], f32, name=f"pxw{s}_{mc}", tag="pvec")
            nc.tensor.matmul(pxw[:, :], w3Tf[:, mc, :], xw_pre[:, :], start=True, stop=False)
            nc.tensor.matmul(pxw[:, :], b3rf[0:1, 128 * mc:128 * (mc + 1)], sx_row[:, :],
                             start=False, stop=True)
            nc.scalar.activation(out=xcat[:, mc, W:2 * W], in_=pxw[:, :], func=AF.Copy,
                                 scale=cm3[:, mc:mc + 1])
        py1 = pvec.tile([MIP, 2 * W], f32, name=f"py1{s}", tag="pvec")
        for k in range(KC):
            nc.tensor.matmul(py1[:, :], caw1[:, k, :], xcat[:, k, :],
                             start=(k == 0), stop=(k == KC - 1))
        r6 = sm2.tile([MIP, 2 * W], f32, name=f"r6_{s}", tag="r6_")
        nc.scalar.activation(out=r6, in_=py1[:, :], func=AF.Relu, bias=cab1p3[:, :])
        r6b = sm2.tile([MIP, 2 * W], f32, name=f"r6b{s}", tag="r6b")
        nc.vector.tensor_scalar(out=r6b, in0=r6, scalar1=6.0, scalar2=1.0 / 6.0,
                                op0=OP.min, op1=OP.mult)
        y1 = sm2.tile([MIP, 2 * W], f32, name=f"y1_{s}", tag="y1_")
        nc.vector.tensor_tensor(out=y1, in0=r6b, in1=py1[:, :], op=OP.mult)
        ahc = sm2.tile([128, MC, W], f32, name=f"ahc{s}", tag="ahc")
        awt = sm2.tile([128, MC, W], f32, name=f"awt{s}", tag="awt")

        if KSTAGE < 7:
            continue
        # ------- stage I: conv3 + CA apply + residual + relu + store -------
        # (per-mc CA vectors interleave with the per-mc conv3 blocks so the
        # first conv3 matmuls start as soon as mc=0's ah/aw are ready)
        for mc in range(MC):
            pah = pvec.tile([128, W], f32, name=f"pah{s}_{mc}", tag="pvec")
            nc.tensor.matmul(pah[:, :], cawh[:, mc, :], y1[:, 0:W], start=True, stop=True)
            aht = sm2.tile([128, W], f32, name=f"aht{s}_{mc}", tag="aht")
            nc.scalar.activation(out=aht, in_=pah[:, :], func=AF.Sigmoid,
                                 bias=cabh[:, mc:mc + 1])
            nc.vector.tensor_scalar(out=ahc[:, mc, :], in0=aht, scalar1=cm3[:, mc:mc + 1],
                                    scalar2=None, op0=OP.mult)
            paw = pvec.tile([128, W], f32, name=f"paw{s}_{mc}", tag="pvec")
            nc.tensor.matmul(paw[:, :], caww[:, mc, :], y1[:, W:2 * W], start=True, stop=True)
            nc.scalar.activation(out=awt[:, mc, :], in_=paw[:, :], func=AF.Sigmoid,
                                 bias=cabw[:, mc:mc + 1])
            for ht in range(NIT):
                osb = outp.tile([128, IW], f32, name=f"osb{s}_{mc}_{ht}", tag="osb")
                pt = pc3.tile([128, 1024], f32, name=f"pt{s}_{mc}_{ht}", tag="pc3")
                for hh in range(2):
                    lo = IW * ht + IH * hh
                    nc.tensor.matmul(pt[:, 512 * hh:512 * hh + IH], w3s[:, mc, :],
                                     h2m[:, lo:lo + IH], start=True, stop=False)
                    nc.tensor.matmul(pt[:, 512 * hh:512 * hh + IH],
                                     b3r[0:1, 128 * mc:128 * (mc + 1)],
                                     signmask[0:1, lo:lo + IH], start=False, stop=True)
                ptv = pt[:, 0:1024].rearrange("p (h z) -> p h z", h=2)[:, :, 0:IH] \
                    .rearrange("p h (y x) -> p h y x", y=IR)
                ahs = ahc[:, mc, 2 * IR * ht:2 * IR * (ht + 1)] \
                    .rearrange("p (h y) -> p h y", h=2).unsqueeze(3) \
                    .broadcast_to([128, 2, IR, W])
                aws = awt[:, mc, :].unsqueeze(1).unsqueeze(1) \
                    .broadcast_to([128, 2, IR, W])
                ut = mid.tile([128, IW], f32, name=f"ut{s}_{mc}_{ht}", tag="ut")
                utv = ut.rearrange("p (h y x) -> p h y x", h=2, y=IR)
                nc.vector.tensor_tensor(out=utv, in0=ptv, in1=ahs, op=OP.mult)
                nc.vector.tensor_tensor(out=ptv, in0=utv, in1=aws, op=OP.mult)
                for hh in range(2):
                    lo = IW * ht + IH * hh
                    nc.tensor.matmul(pt[:, 512 * hh:512 * hh + IH], idn[:, :],
                                     xkr[mc][:, lo:lo + IH], start=False, stop=True,
                                     skip_group_check=True)
                nc.scalar.activation(
                    out=osb.rearrange("p (h y x) -> p h y x", h=2, y=IR),
                    in_=ptv, func=AF.Relu)
                oeng = nc.sync if (NIT * mc + ht) % 2 == 0 else nc.scalar
                oeng.dma_start(
                    out=out_d[s, 128 * mc:128 * (mc + 1), :, :].rearrange(
                        "c h w -> c (h w)")[:, IW * ht:IW * (ht + 1)],
                    in_=osb[:, :])


def _build():
    nc = bacc.Bacc("TRN2", target_bir_lowering=False, debug=False)
    d = {
        "x": nc.dram_tensor("x", [BS, 512, H, W], f32, kind="ExternalInput"),
        "wbr": nc.dram_tensor("wbr", [128, CR], f32r, kind="ExternalInput"),
        "wbf": nc.dram_tensor("wbf", [128, CF], f32, kind="ExternalInput"),
        "out": nc.dram_tensor("out", [BS, 512, H, W], f32, kind="ExternalOutput"),
    }
    with tile.TileContext(nc) as tc, ExitStack() as ctx:
        _emit(nc, tc, ctx, d)
    nc.compile()
    return nc


def _prep_weights(i):
    """Host-side rearrangement of the full (replicated) weights into the two
    packed SBUF blobs."""
    N = NPIX
    w1 = i["w1"][:, :, 0, 0]                     # [128, 512]
    w3 = i["w3"][:, :, 0, 0]                     # [512, 128]
    caw1 = i["ca_w1"][:, :, 0, 0]                # [16, 512]
    cawh = i["ca_wh"][:, :, 0, 0]                # [512, 16]
    caww = i["ca_ww"][:, :, 0, 0]                # [512, 16]
    smv = i["sm_w"][0, :, 0, 0]                  # [512]
    f = np.float32
    w3T = np.ascontiguousarray(w3.T.reshape(128, MC, 128), f)
    arrs = {
        "w1T": np.ascontiguousarray(w1.T.reshape(KC, 128, 128).transpose(1, 0, 2), f),
        "w2T": np.ascontiguousarray(i["w2"].transpose(1, 2, 3, 0).reshape(128, 9, 128), f),
        "w3T": w3T,
        "w3Tf": w3T,
        "b3rf": i["b3"].reshape(1, 512).astype(f),
        "smw": np.ascontiguousarray(smv.reshape(KC, 128).T, f),
        "cm1w": np.ascontiguousarray(
            (i["cm1_w"] / N).T.reshape(KC, 128, 128).transpose(1, 0, 2), f),
        "cm2w": np.ascontiguousarray((i["cm2_w"] / N).T, f),
        "cm3w": np.ascontiguousarray((i["cm3_w"] / N).T.reshape(128, MC, 128), f),
        "caw1": np.ascontiguousarray(
            (caw1 / W).T.reshape(KC, 128, MIP).transpose(1, 0, 2), f),
        "cawh": np.ascontiguousarray(cawh.T.reshape(MIP, MC, 128), f),
        "caww": np.ascontiguousarray(caww.T.reshape(MIP, MC, 128), f),
        "idn": np.eye(128, dtype=f),
        "b1v": i["b1"].reshape(128, 1).astype(f),
        "b2v": i["b2"].reshape(128, 1).astype(f),
        "b3r": i["b3"].reshape(1, 512).astype(f),
        "smbneg": np.full((128, 1), -i["sm_b"][0], f),
        "cm1nb": (-i["cm1_b"]).reshape(128, 1).astype(f),
        "cm2nb": (-i["cm2_b"]).reshape(128, 1).astype(f),
        "cm3nb": np.ascontiguousarray((-i["cm3_b"]).reshape(MC, 128).T, f),
        "cab1p3": (i["ca_b1"] + 3.0).reshape(MIP, 1).astype(f),
        "cabh": np.ascontiguousarray(i["ca_bh"].reshape(MC, 128).T, f),
        "cabw": np.ascontiguousarray(i["ca_bw"].reshape(MC, 128).T, f),
    }

    def pack(layout, cols):
        blob = np.zeros((128, cols), f)
        off = 0
        for name, shp in layout:
            n = int(np.prod(shp[1:]))
            if name != "zpad":
                blob[0:shp[0], off:off + n] = arrs[name].reshape(shp[0], n)
            off += n
        return blob

    return {"wbr": pack(WBR_LAYOUT, CR), "wbf": pack(WBF_LAYOUT, CF)}


_NC_CACHE = None


def _get_nc():
    global _NC_CACHE
    if _NC_CACHE is None:
        _NC_CACHE = _build()
    return _NC_CACHE


def kernel(**inputs):
    nc = _get_nc()
    wmap = _prep_weights(inputs)
    x = np.ascontiguousarray(inputs["x"], np.float32)
    in_maps = []
    for c in range(NCORES):
        m = dict(wmap)
        m["x"] = np.ascontiguousarray(x[BS * c:BS * (c + 1)])
        in_maps.append(m)
    res = run_bass_kernel_spmd(nc, in_maps, core_ids=list(range(NCORES)))
    return np.concatenate([r["out"] for r in res.results], axis=0)
